# revision 1
# baseline (speedup 1.0000x reference)
"""Multi-head causal self-attention (B=2, T=2048, D=1024, H=16) on 8 trn2 cores.

Sharding: data-parallel over batch (cores 0-3 -> batch 0, 4-7 -> batch 1),
tensor-parallel over heads within each 4-core group (4 heads per core).
Wq/Wk/Wv column-sharded, Wo row-sharded; partial outputs reduce-scattered
on device within each 4-core group; host concatenates the row shards.

Per-core pipeline (all matmuls fp32r):
  x [2048,1024] --PE transpose--> xT [1024,2048]
  qT/kT = W_slice @ x.T   (heads on partitions, 2-head pairs stacked 128-wide)
  v     = x @ Wv_slice.T  (natural layout, +ones column for softmax denom)
  per (head-pair, 512-query block): stream 128-key tiles:
     scoresT = kT.T-chunk x qT   (two row-packed K=64 matmuls, psum [128k,512q])
     expT = exp(0.125*scoresT)   (ACT, psum->sbuf fp32r)
     causal mask on diagonal tiles (gpsimd affine_select, fill 0)
     out_augT += v_aug.T @ expT  (psum [65,512]: rows 0-63 att, row 64 denom)
  normalize: bcast denom reciprocal via ones-outer-product matmul, DVE divide
  out_partial = attT.T @ WoT    (K=128 chunks)
  ReduceScatter(add) over the 4-core group -> [512,1024] rows per core
"""

import sys

for _p in ("/opt/trn_rl_repo", "/root/.axon_site/_ro/trn_rl_repo"):
    if _p not in sys.path:
        sys.path.append(_p)

import numpy as np

import concourse.bass as bass
import concourse.mybir as mybir
import concourse.tile as tile
from concourse import bacc
from concourse.bass_utils import run_bass_kernel_spmd
from concourse.masks import make_identity

F32 = mybir.dt.float32
F32R = mybir.dt.float32r

B, T, D = 2, 2048, 1024
H, DH = 16, 64
HPC = 4          # heads per core
FPC = HPC * DH   # feature dims per core (256)
NKT = T // 128   # 16 key tiles / token tiles
NQB = T // 512   # 4 query blocks
VW = DH + 1      # v width incl ones column (65)

_CACHE = {}


def _build():
    nc = bacc.Bacc("TRN2", target_bir_lowering=False, debug=False, num_devices=8)

    x_d = nc.dram_tensor("x", [T, D], F32, kind="ExternalInput").ap()
    wq_d = nc.dram_tensor("wq_t", [D, FPC], F32, kind="ExternalInput").ap()
    wk_d = nc.dram_tensor("wk_t", [D, FPC], F32, kind="ExternalInput").ap()
    wv_d = nc.dram_tensor("wv_t", [D, FPC], F32, kind="ExternalInput").ap()
    wo_d = nc.dram_tensor("wo_t", [FPC, D], F32, kind="ExternalInput").ap()
    ones_d = nc.dram_tensor("ones_c", [128, 64], F32, kind="ExternalInput").ap()
    out_d = nc.dram_tensor("out", [T // 4, D], F32, kind="ExternalOutput").ap()

    po = nc.dram_tensor("po", [T, D], F32)           # partial output (pre-reduce)
    rs_out = nc.dram_tensor("rs_out", [T // 4, D], F32)

    with tile.TileContext(nc) as tc:
        with (
            tc.tile_pool(name="wp", bufs=1) as wp,
            tc.tile_pool(name="qk", bufs=1) as qk,
            tc.tile_pool(name="vp", bufs=1) as vp,
            tc.tile_pool(name="at", bufs=1) as at,
        ):
            # ---- persistent SBUF ----
            wq_sb = wp.tile([128, 8 * FPC], F32R)
            nc.sync.dma_start(
                wq_sb[:].rearrange("p (c f) -> p c f", f=FPC),
                wq_d.rearrange("(c p) f -> p c f", p=128).bitcast(F32R),
            )
            wk_sb = wp.tile([128, 8 * FPC], F32R)
            nc.sync.dma_start(
                wk_sb[:].rearrange("p (c f) -> p c f", f=FPC),
                wk_d.rearrange("(c p) f -> p c f", p=128).bitcast(F32R),
            )
            wv_sb = wp.tile([128, 8 * FPC], F32R)
            nc.sync.dma_start(
                wv_sb[:].rearrange("p (c f) -> p c f", f=FPC),
                wv_d.rearrange("(c p) f -> p c f", p=128).bitcast(F32R),
            )
            wo_sb = wp.tile([128, 2 * D], F32R)
            nc.sync.dma_start(
                wo_sb[:].rearrange("p (c f) -> p c f", f=D),
                wo_d.rearrange("(c p) f -> p c f", p=128).bitcast(F32R),
            )
            ones_sb = wp.tile([128, 64], F32R)
            nc.sync.dma_start(ones_sb[:], ones_d.bitcast(F32R))
            ident = wp.tile([128, 128], F32)
            make_identity(nc, ident[:])

            qT_sb = qk.tile([128, 2 * T], F32R)   # head-pair hp at cols hp*T
            kT_sb = qk.tile([128, 2 * T], F32R)
            v_sb = vp.tile([128, NKT * HPC * VW], F32R)  # tok-tile tt, head h at (tt*4+h)*65
            attT_sb = at.tile([128, 2 * T], F32R)

            # ones columns of v (every 65th col, offset 64)
            nc.sync.dma_start(
                v_sb[:].rearrange("p (a b) -> p a b", b=VW)[:, :, 64],
                ones_d[:, 0 : NKT * HPC].bitcast(F32R),
            )

            # ---- phase 1+2: transpose x, projections ----
            with (
                tc.tile_pool(name="xt", bufs=1) as xtp,
                tc.tile_pool(name="xn", bufs=3) as xnp,
                tc.tile_pool(name="ps12", bufs=1, space="PSUM") as ps12,
            ):
                xT_sb = xtp.tile([128, 8 * T], F32R)  # dm chunk kc at cols kc*T
                for tt in range(NKT):
                    x_t = xnp.tile([128, D], F32)
                    nc.sync.dma_start(x_t[:], x_d[tt * 128 : (tt + 1) * 128, :])
                    for kc in range(8):
                        tp_ps = ps12.tile([128, 128], F32, tag="tp", bufs=3)
                        nc.tensor.transpose(
                            tp_ps[:], x_t[:, kc * 128 : (kc + 1) * 128], ident[:]
                        )
                        nc.vector.tensor_copy(
                            xT_sb[:, kc * T + tt * 128 : kc * T + (tt + 1) * 128],
                            tp_ps[:],
                        )

                # qT / kT projections: [feat(128=2 heads), tok] blocks
                for hp in range(2):
                    for tb in range(NQB):
                        q_ps = ps12.tile([128, 512], F32, tag="proj", bufs=2)
                        k_ps = ps12.tile([128, 512], F32, tag="proj", bufs=2)
                        for kc in range(8):
                            nc.tensor.matmul(
                                q_ps[:],
                                wq_sb[:, kc * FPC + hp * 128 : kc * FPC + (hp + 1) * 128],
                                xT_sb[:, kc * T + tb * 512 : kc * T + (tb + 1) * 512],
                                start=(kc == 0), stop=(kc == 7),
                            )
                        for kc in range(8):
                            nc.tensor.matmul(
                                k_ps[:],
                                wk_sb[:, kc * FPC + hp * 128 : kc * FPC + (hp + 1) * 128],
                                xT_sb[:, kc * T + tb * 512 : kc * T + (tb + 1) * 512],
                                start=(kc == 0), stop=(kc == 7),
                            )
                        nc.vector.tensor_copy(
                            qT_sb[:, hp * T + tb * 512 : hp * T + (tb + 1) * 512], q_ps[:]
                        )
                        nc.vector.tensor_copy(
                            kT_sb[:, hp * T + tb * 512 : hp * T + (tb + 1) * 512], k_ps[:]
                        )

                # v projection: natural [tok, feat] tiles
                for tt in range(NKT):
                    v_ps = ps12.tile([128, FPC], F32, tag="vproj", bufs=2)
                    for kc in range(8):
                        nc.tensor.matmul(
                            v_ps[:],
                            xT_sb[:, kc * T + tt * 128 : kc * T + (tt + 1) * 128],
                            wv_sb[:, kc * FPC : (kc + 1) * FPC],
                            start=(kc == 0), stop=(kc == 7),
                        )
                    nc.vector.tensor_copy(
                        v_sb[:].rearrange("p (a b) -> p a b", b=VW)[
                            :, tt * HPC : (tt + 1) * HPC, 0:DH
                        ],
                        v_ps[:].rearrange("p (a b) -> p a b", b=DH),
                    )

            # ---- phase 3: attention ----
            with (
                tc.tile_pool(name="ep", bufs=3) as ep,
                tc.tile_pool(name="nr", bufs=2) as nrm,
                tc.tile_pool(name="ps3", bufs=1, space="PSUM") as ps3,
            ):
                for hp in range(2):
                    hA, hB = 2 * hp, 2 * hp + 1
                    for qb in range(NQB):
                        oA = ps3.tile([VW, 512], F32, tag="oA", bufs=1)
                        oB = ps3.tile([VW, 512], F32, tag="oB", bufs=1)
                        nkt = 4 * (qb + 1)
                        for kt in range(nkt):
                            sA = ps3.tile([128, 512], F32, tag="sA", bufs=2)
                            sB = ps3.tile([128, 512], F32, tag="sB", bufs=2)
                            nc.tensor.matmul(
                                sA[:],
                                kT_sb[0:64, hp * T + kt * 128 : hp * T + (kt + 1) * 128],
                                qT_sb[0:64, hp * T + qb * 512 : hp * T + (qb + 1) * 512],
                                start=True, stop=True, tile_position=(0, 0),
                            )
                            nc.tensor.matmul(
                                sB[:],
                                kT_sb[64:128, hp * T + kt * 128 : hp * T + (kt + 1) * 128],
                                qT_sb[64:128, hp * T + qb * 512 : hp * T + (qb + 1) * 512],
                                start=True, stop=True, tile_position=(64, 0),
                            )
                            eA = ep.tile([128, 512], F32R, tag="eA")
                            eB = ep.tile([128, 512], F32R, tag="eB")
                            nc.scalar.activation(
                                eA[:], sA[:], mybir.ActivationFunctionType.Exp,
                                scale=0.125,
                            )
                            nc.scalar.activation(
                                eB[:], sB[:], mybir.ActivationFunctionType.Exp,
                                scale=0.125,
                            )
                            base = qb * 512 - kt * 128
                            if base <= 0:  # diagonal tile: mask k > q
                                for e in (eA, eB):
                                    nc.gpsimd.affine_select(
                                        out=e[:], in_=e[:],
                                        pattern=[[1, 512]],
                                        compare_op=mybir.AluOpType.is_ge,
                                        fill=0.0, base=base, channel_multiplier=-1,
                                    )
                            nc.tensor.matmul(
                                oA[:],
                                v_sb[:, (kt * HPC + hA) * VW : (kt * HPC + hA + 1) * VW],
                                eA[:],
                                start=(kt == 0), stop=(kt == nkt - 1),
                            )
                            nc.tensor.matmul(
                                oB[:],
                                v_sb[:, (kt * HPC + hB) * VW : (kt * HPC + hB + 1) * VW],
                                eB[:],
                                start=(kt == 0), stop=(kt == nkt - 1),
                            )
                        # normalize: attT[:, q] = att[:, q] / denom[q]
                        for o_ps, prow in ((oA, 0), (oB, 64)):
                            srow = nrm.tile([1, 512], F32R, tag="srow")
                            nc.scalar.copy(srow[:], o_ps[64:65, :])
                            bc_ps = ps3.tile([64, 512], F32, tag="bc", bufs=2)
                            nc.tensor.matmul(
                                bc_ps[:], ones_sb[0:1, 0:64], srow[:],
                                start=True, stop=True,
                            )
                            rec = nrm.tile([64, 512], F32, tag="rec")
                            nc.vector.reciprocal(rec[:], bc_ps[:])
                            nc.vector.tensor_mul(
                                attT_sb[
                                    prow : prow + 64,
                                    hp * T + qb * 512 : hp * T + (qb + 1) * 512,
                                ],
                                o_ps[0:64, :],
                                rec[:],
                            )

            # ---- phase 4: output projection ----
            with (
                tc.tile_pool(name="op", bufs=3) as op,
                tc.tile_pool(name="ps4", bufs=1, space="PSUM") as ps4,
            ):
                for tt in range(NKT):
                    o_sb = op.tile([128, D], F32, tag="osb")
                    for nck in range(2):
                        wo_ps = ps4.tile([128, 512], F32, tag="wo", bufs=3)
                        for hp in range(2):
                            nc.tensor.matmul(
                                wo_ps[:],
                                attT_sb[:, hp * T + tt * 128 : hp * T + (tt + 1) * 128],
                                wo_sb[:, hp * D + nck * 512 : hp * D + (nck + 1) * 512],
                                start=(hp == 0), stop=(hp == 1),
                            )
                        nc.vector.tensor_copy(
                            o_sb[:, nck * 512 : (nck + 1) * 512], wo_ps[:]
                        )
                    nc.sync.dma_start(po[tt * 128 : (tt + 1) * 128, :], o_sb[:])

            # ---- phase 5: reduce-scatter within 4-core group ----
            nc.gpsimd.collective_compute(
                "ReduceScatter",
                mybir.AluOpType.add,
                replica_groups=[[0, 1, 2, 3], [4, 5, 6, 7]],
                ins=[po[:]],
                outs=[rs_out[:]],
            )
            nc.sync.dma_start(out_d[:], rs_out[:])

    nc.compile()
    return nc


def _prep_in_maps(x, Wq, Wk, Wv, Wo):
    x = np.asarray(x, dtype=np.float32)
    Wq = np.asarray(Wq, dtype=np.float32)
    Wk = np.asarray(Wk, dtype=np.float32)
    Wv = np.asarray(Wv, dtype=np.float32)
    Wo = np.asarray(Wo, dtype=np.float32)
    ones_c = np.ones((128, 64), dtype=np.float32)
    in_maps = []
    for c in range(8):
        b, g = divmod(c, 4)
        sl = slice(g * FPC, (g + 1) * FPC)
        in_maps.append(
            {
                "x": np.ascontiguousarray(x[b]),
                "wq_t": np.ascontiguousarray(Wq[sl, :].T),
                "wk_t": np.ascontiguousarray(Wk[sl, :].T),
                "wv_t": np.ascontiguousarray(Wv[sl, :].T),
                "wo_t": np.ascontiguousarray(Wo[:, sl].T),
                "ones_c": ones_c,
            }
        )
    return in_maps


def _get_nc():
    if "nc" not in _CACHE:
        _CACHE["nc"] = _build()
    return _CACHE["nc"]


def _assemble(results):
    out = np.empty((B, T, D), dtype=np.float32)
    for b in range(B):
        out[b] = np.concatenate(
            [results[4 * b + r]["out"] for r in range(4)], axis=0
        )
    return out


def kernel(x, Wq, Wk, Wv, Wo):
    nc = _get_nc()
    in_maps = _prep_in_maps(x, Wq, Wk, Wv, Wo)
    res = run_bass_kernel_spmd(nc, in_maps, core_ids=list(range(8)))
    return _assemble(res.results)


def kernel_with_trace(x, Wq, Wk, Wv, Wo, **kw):
    nc = _get_nc()
    in_maps = _prep_in_maps(x, Wq, Wk, Wv, Wo)
    res = run_bass_kernel_spmd(nc, in_maps, core_ids=list(range(8)), trace=True, **kw)
    return _assemble(res.results), res


# revision 8
# speedup vs baseline: 2.1353x; 2.1353x over previous
"""Multi-head causal self-attention (B=2, T=2048, D=1024, H=16) on 8 trn2 cores.

Sharding: data-parallel over batch (cores 0-3 -> batch 0, 4-7 -> batch 1),
tensor-parallel over heads within each 4-core group (4 heads per core).
Wq/Wk/Wv column-sharded, Wo row-sharded; each core emits its partial output
projection and the host sums the 4 partials per batch (TP unshard).

Per-core pipeline (bf16 matmul operands, fp32 PSUM accumulation):
  x [2048,1024] -> bf16 -> PE transpose -> xT [1024,2048]
  qT/kT = W_slice @ x.T   (heads on partitions, 2-head pairs stacked 128-wide)
  v     = x @ Wv_slice.T  (natural layout, +ones column for softmax denom)
  per (512-query block, head-pair): stream 128-key tiles:
     scoresT pair -> one 2-bank psum tile [128k, 2head*512q] (row-packed K=64 matmuls)
     expT = exp(0.125*scoresT)  (single ACT call over both heads, psum->sbuf bf16)
     causal mask on diagonal tiles (gpsimd affine_select, fill 0)
     out_augT += v_aug.T @ expT (psum [65,512]: rows 0-63 att, row 64 denom)
  normalize per (qb,hp): denom rows lane-packed via sbuf DMA for parallel
  reciprocal, partition-broadcast via DMA, single DVE mul psum->attT (bf16)
  out_partial(qb) = attT.T @ WoT interleaved with next query block's attention
"""

import sys

for _p in ("/opt/trn_rl_repo", "/root/.axon_site/_ro/trn_rl_repo"):
    if _p not in sys.path:
        sys.path.append(_p)

import ml_dtypes
import numpy as np

import concourse.bass as bass
import concourse.mybir as mybir
import concourse.tile as tile
from concourse import bacc
from concourse.bass_utils import run_bass_kernel_spmd
from concourse.masks import make_identity

F32 = mybir.dt.float32
BF16 = mybir.dt.bfloat16

B, T, D = 2, 2048, 1024
H, DH = 16, 64
HPC = 4          # heads per core
FPC = HPC * DH   # feature dims per core (256)
NKT = T // 128   # 16 key tiles / token tiles
NQB = T // 512   # 4 query blocks
VW = DH + 1      # v width incl ones column (65)

_CACHE = {}


def _build():
    nc = bacc.Bacc("TRN2", target_bir_lowering=False, debug=False, num_devices=8)

    x_d = nc.dram_tensor("x", [T, D], F32, kind="ExternalInput").ap()
    wq_d = nc.dram_tensor("wq_t", [D, FPC], BF16, kind="ExternalInput").ap()
    wk_d = nc.dram_tensor("wk_t", [D, FPC], BF16, kind="ExternalInput").ap()
    wv_d = nc.dram_tensor("wv_t", [D, FPC], BF16, kind="ExternalInput").ap()
    wo_d = nc.dram_tensor("wo_t", [FPC, D], BF16, kind="ExternalInput").ap()
    onesb_d = nc.dram_tensor("ones_b", [128, 64], BF16, kind="ExternalInput").ap()
    out_d = nc.dram_tensor("po", [T, D], F32, kind="ExternalOutput").ap()
    rscr_d = nc.dram_tensor("rscr", [8, 1024], F32).ap()

    with tile.TileContext(nc) as tc:
        with (
            tc.tile_pool(name="wp", bufs=1) as wp,
            tc.tile_pool(name="qk", bufs=1) as qk,
            tc.tile_pool(name="vp", bufs=1) as vp,
            tc.tile_pool(name="at", bufs=1) as at,
        ):
            ident = wp.tile([128, 128], BF16)
            make_identity(nc, ident[:])

            qT_sb = qk.tile([128, 2 * T], BF16)   # head-pair hp at cols hp*T
            kT_sb = qk.tile([128, 2 * T], BF16)
            v_sb = vp.tile([128, NKT * HPC * VW], BF16)
            attT_sb = at.tile([128, 2 * T], BF16)

            # ---- phase 1+2: transpose x, projections ----
            with (
                tc.tile_pool(name="xt", bufs=1) as xtp,
                tc.tile_pool(name="xn", bufs=3) as xnp,
                tc.tile_pool(name="ps12", bufs=1, space="PSUM") as ps12,
            ):
                xT_sb = xtp.tile([128, 8 * T], BF16)  # dm chunk kc at cols kc*T
                for tt in range(NKT):
                    x_t = xnp.tile([128, D], F32, tag="x_t")
                    nc.sync.dma_start(x_t[:], x_d[tt * 128 : (tt + 1) * 128, :])
                    xb_t = xnp.tile([128, D], BF16, tag="xb_t")
                    nc.vector.tensor_copy(xb_t[:], x_t[:])
                    for kc in range(8):
                        tp_ps = ps12.tile([128, 128], BF16, tag="tp", bufs=3)
                        nc.tensor.transpose(
                            tp_ps[:], xb_t[:, kc * 128 : (kc + 1) * 128], ident[:]
                        )
                        nc.vector.tensor_copy(
                            xT_sb[:, kc * T + tt * 128 : kc * T + (tt + 1) * 128],
                            tp_ps[:],
                        )

                # weights (needed later than x, so DMA'd after)
                wq_sb = wp.tile([128, 8 * FPC], BF16)
                nc.sync.dma_start(
                    wq_sb[:].rearrange("p (c f) -> p c f", f=FPC),
                    wq_d.rearrange("(c p) f -> p c f", p=128),
                )
                wk_sb = wp.tile([128, 8 * FPC], BF16)
                nc.sync.dma_start(
                    wk_sb[:].rearrange("p (c f) -> p c f", f=FPC),
                    wk_d.rearrange("(c p) f -> p c f", p=128),
                )
                wv_sb = wp.tile([128, 8 * FPC], BF16)
                nc.sync.dma_start(
                    wv_sb[:].rearrange("p (c f) -> p c f", f=FPC),
                    wv_d.rearrange("(c p) f -> p c f", p=128),
                )
                wo_sb = wp.tile([128, 2 * D], BF16)
                nc.sync.dma_start(
                    wo_sb[:].rearrange("p (c f) -> p c f", f=D),
                    wo_d.rearrange("(c p) f -> p c f", p=128),
                )
                # ones columns of v (every 65th col, offset 64)
                nc.sync.dma_start(
                    v_sb[:].rearrange("p (a b) -> p a b", b=VW)[:, :, 64],
                    onesb_d[:, 0 : NKT * HPC],
                )

                # qT / kT projections: [feat(128=2 heads), tok] blocks
                for hp in range(2):
                    for tb in range(NQB):
                        q_ps = ps12.tile([128, 512], F32, tag="proj", bufs=2)
                        k_ps = ps12.tile([128, 512], F32, tag="proj", bufs=2)
                        for kc in range(8):
                            nc.tensor.matmul(
                                q_ps[:],
                                wq_sb[:, kc * FPC + hp * 128 : kc * FPC + (hp + 1) * 128],
                                xT_sb[:, kc * T + tb * 512 : kc * T + (tb + 1) * 512],
                                start=(kc == 0), stop=(kc == 7),
                            )
                        for kc in range(8):
                            nc.tensor.matmul(
                                k_ps[:],
                                wk_sb[:, kc * FPC + hp * 128 : kc * FPC + (hp + 1) * 128],
                                xT_sb[:, kc * T + tb * 512 : kc * T + (tb + 1) * 512],
                                start=(kc == 0), stop=(kc == 7),
                            )
                        nc.vector.tensor_copy(
                            qT_sb[:, hp * T + tb * 512 : hp * T + (tb + 1) * 512], q_ps[:]
                        )
                        nc.vector.tensor_copy(
                            kT_sb[:, hp * T + tb * 512 : hp * T + (tb + 1) * 512], k_ps[:]
                        )

                # v projection: natural [tok, feat] tiles
                for tt in range(NKT):
                    v_ps = ps12.tile([128, FPC], F32, tag="vproj", bufs=2)
                    for kc in range(8):
                        nc.tensor.matmul(
                            v_ps[:],
                            xT_sb[:, kc * T + tt * 128 : kc * T + (tt + 1) * 128],
                            wv_sb[:, kc * FPC : (kc + 1) * FPC],
                            start=(kc == 0), stop=(kc == 7),
                        )
                    nc.vector.tensor_copy(
                        v_sb[:].rearrange("p (a b) -> p a b", b=VW)[
                            :, tt * HPC : (tt + 1) * HPC, 0:DH
                        ],
                        v_ps[:].rearrange("p (a b) -> p a b", b=DH),
                    )

            # ---- phase 3: attention + per-block output projection ----
            with (
                tc.tile_pool(name="ep", bufs=4) as ep,
                tc.tile_pool(name="nr", bufs=2) as nrm,
                tc.tile_pool(name="op", bufs=3) as op,
                tc.tile_pool(name="ps3", bufs=1, space="PSUM") as ps3,
            ):
                for qb in range(NQB):
                    for hp in range(2):
                        hA, hB = 2 * hp, 2 * hp + 1
                        oA = ps3.tile([VW, 512], F32, tag="oA", bufs=2)
                        oB = ps3.tile([VW, 512], F32, tag="oB", bufs=2)
                        nkt = 4 * (qb + 1)
                        for kt in range(nkt):
                            sAB = ps3.tile([128, 1024], F32, tag="sAB", bufs=2)
                            nc.tensor.matmul(
                                sAB[:, 0:512],
                                kT_sb[0:64, hp * T + kt * 128 : hp * T + (kt + 1) * 128],
                                qT_sb[0:64, hp * T + qb * 512 : hp * T + (qb + 1) * 512],
                                start=True, stop=True, tile_position=(0, 0),
                            )
                            nc.tensor.matmul(
                                sAB[:, 512:1024],
                                kT_sb[64:128, hp * T + kt * 128 : hp * T + (kt + 1) * 128],
                                qT_sb[64:128, hp * T + qb * 512 : hp * T + (qb + 1) * 512],
                                start=True, stop=True, tile_position=(64, 0),
                            )
                            eAB = ep.tile([128, 1024], BF16, tag="eAB")
                            nc.scalar.activation(
                                eAB[:], sAB[:], mybir.ActivationFunctionType.Exp,
                                scale=0.125,
                            )
                            base = qb * 512 - kt * 128
                            if base <= 0:  # diagonal tile: mask k > q
                                nc.gpsimd.affine_select(
                                    out=eAB[:].rearrange("p (h q) -> p h q", q=512),
                                    in_=eAB[:].rearrange("p (h q) -> p h q", q=512),
                                    pattern=[[0, 2], [1, 512]],
                                    compare_op=mybir.AluOpType.is_ge,
                                    fill=0.0, base=base, channel_multiplier=-1,
                                )
                            nc.tensor.matmul(
                                oA[:],
                                v_sb[:, (kt * HPC + hA) * VW : (kt * HPC + hA + 1) * VW],
                                eAB[:, 0:512],
                                start=(kt == 0), stop=(kt == nkt - 1),
                            )
                            nc.tensor.matmul(
                                oB[:],
                                v_sb[:, (kt * HPC + hB) * VW : (kt * HPC + hB + 1) * VW],
                                eAB[:, 512:1024],
                                start=(kt == 0), stop=(kt == nkt - 1),
                            )
                        # normalize (qb, hp): pack denoms, reciprocal, bcast, mul
                        srows = nrm.tile([1, 1024], F32, tag="srows")
                        nc.vector.tensor_copy(srows[0:1, 0:512], oA[64:65, :])
                        nc.vector.tensor_copy(srows[0:1, 512:1024], oB[64:65, :])
                        packed = nrm.tile([128, 8], F32, tag="packed")
                        nc.sync.dma_start(
                            packed[:],
                            srows[:].rearrange("r (g e) -> r g e", e=8),
                        )
                        rpacked = nrm.tile([128, 8], F32, tag="rpacked")
                        nc.vector.reciprocal(rpacked[:], packed[:])
                        ridx = qb * 2 + hp
                        rrow_d = rscr_d[ridx : ridx + 1, :]
                        nc.sync.dma_start(
                            rrow_d.rearrange("r (g e) -> r g e", e=8),
                            rpacked[:],
                        )
                        for o_ps, prow, off in ((oA, 0, 0), (oB, 64, 512)):
                            bc = nrm.tile([64, 512], F32, tag="bc")
                            nc.sync.dma_start(
                                bc[:],
                                rrow_d[0:1, off : off + 512].partition_broadcast(64),
                            )
                            nc.vector.tensor_mul(
                                attT_sb[
                                    prow : prow + 64,
                                    hp * T + qb * 512 : hp * T + (qb + 1) * 512,
                                ],
                                o_ps[0:64, :],
                                bc[:],
                            )
                    # output projection for this query block's 4 token tiles
                    for t4 in range(4):
                        tt = qb * 4 + t4
                        o_sb = op.tile([128, D], F32, tag="osb")
                        for nck in range(2):
                            wo_ps = ps3.tile(
                                [128, 512], F32,
                                tag=("oA" if nck == 0 else "oB"), bufs=2,
                            )
                            for hp in range(2):
                                nc.tensor.matmul(
                                    wo_ps[:],
                                    attT_sb[:, hp * T + tt * 128 : hp * T + (tt + 1) * 128],
                                    wo_sb[:, hp * D + nck * 512 : hp * D + (nck + 1) * 512],
                                    start=(hp == 0), stop=(hp == 1),
                                )
                            nc.vector.tensor_copy(
                                o_sb[:, nck * 512 : (nck + 1) * 512], wo_ps[:]
                            )
                        nc.sync.dma_start(out_d[tt * 128 : (tt + 1) * 128, :], o_sb[:])

    nc.compile()
    return nc


def _prep_in_maps(x, Wq, Wk, Wv, Wo):
    x = np.asarray(x, dtype=np.float32)
    bf = ml_dtypes.bfloat16
    Wq = np.asarray(Wq, dtype=np.float32)
    Wk = np.asarray(Wk, dtype=np.float32)
    Wv = np.asarray(Wv, dtype=np.float32)
    Wo = np.asarray(Wo, dtype=np.float32)
    ones_b = np.ones((128, 64), dtype=bf)
    in_maps = []
    for c in range(8):
        b, g = divmod(c, 4)
        sl = slice(g * FPC, (g + 1) * FPC)
        in_maps.append(
            {
                "x": np.ascontiguousarray(x[b]),
                "wq_t": np.ascontiguousarray(Wq[sl, :].T).astype(bf),
                "wk_t": np.ascontiguousarray(Wk[sl, :].T).astype(bf),
                "wv_t": np.ascontiguousarray(Wv[sl, :].T).astype(bf),
                "wo_t": np.ascontiguousarray(Wo[:, sl].T).astype(bf),
                "ones_b": ones_b,
            }
        )
    return in_maps


def _get_nc():
    if "nc" not in _CACHE:
        _CACHE["nc"] = _build()
    return _CACHE["nc"]


def _assemble(results):
    out = np.empty((B, T, D), dtype=np.float32)
    for b in range(B):
        out[b] = (
            results[4 * b]["po"]
            + results[4 * b + 1]["po"]
            + results[4 * b + 2]["po"]
            + results[4 * b + 3]["po"]
        )
    return out


def kernel(x, Wq, Wk, Wv, Wo):
    nc = _get_nc()
    in_maps = _prep_in_maps(x, Wq, Wk, Wv, Wo)
    res = run_bass_kernel_spmd(nc, in_maps, core_ids=list(range(8)))
    return _assemble(res.results)


def kernel_with_trace(x, Wq, Wk, Wv, Wo, **kw):
    nc = _get_nc()
    in_maps = _prep_in_maps(x, Wq, Wk, Wv, Wo)
    res = run_bass_kernel_spmd(nc, in_maps, core_ids=list(range(8)), trace=True, **kw)
    return _assemble(res.results), res


# revision 10
# speedup vs baseline: 2.2657x; 1.0611x over previous
"""Multi-head causal self-attention (B=2, T=2048, D=1024, H=16) on 8 trn2 cores.

Sharding: data-parallel over batch (cores 0-3 -> batch 0, 4-7 -> batch 1),
tensor-parallel over heads within each 4-core group (4 heads per core).
Wq/Wk/Wv column-sharded, Wo row-sharded; each core emits its partial output
projection and the host sums the 4 partials per batch (TP unshard).

Per-core pipeline (bf16 matmul operands, fp32 PSUM accumulation):
  x [2048,1024] -> bf16 -> PE transpose -> xT [1024,2048]
  qT/kT = W_slice @ x.T   (heads on partitions, 2-head pairs stacked 128-wide)
  v     = x @ Wv_slice.T  (natural layout, +ones column for softmax denom)
  per (512-query block, head-pair): stream 128-key tiles:
     scoresT pair -> one 2-bank psum tile [128k, 2head*512q] (row-packed K=64 matmuls)
     expT = exp(0.125*scoresT)  (single ACT call over both heads, psum->sbuf bf16)
     causal mask on diagonal tiles (gpsimd affine_select, fill 0)
     out_augT += v_aug.T @ expT (psum [65,512]: rows 0-63 att, row 64 denom)
  normalize per (qb,hp): denom rows lane-packed via sbuf DMA for parallel
  reciprocal, partition-broadcast via DMA, single DVE mul psum->attT (bf16)
  out_partial(qb) = attT.T @ WoT interleaved with next query block's attention
"""

import sys

for _p in ("/opt/trn_rl_repo", "/root/.axon_site/_ro/trn_rl_repo"):
    if _p not in sys.path:
        sys.path.append(_p)

import ml_dtypes
import numpy as np

import concourse.bass as bass
import concourse.mybir as mybir
import concourse.tile as tile
from concourse import bacc
from concourse.bass_utils import run_bass_kernel_spmd
from concourse.masks import make_identity

F32 = mybir.dt.float32
BF16 = mybir.dt.bfloat16

B, T, D = 2, 2048, 1024
H, DH = 16, 64
HPC = 4          # heads per core
FPC = HPC * DH   # feature dims per core (256)
NKT = T // 128   # 16 key tiles / token tiles
NQB = T // 512   # 4 query blocks
VW = DH + 1      # v width incl ones column (65)

_CACHE = {}


def _build():
    nc = bacc.Bacc("TRN2", target_bir_lowering=False, debug=False, num_devices=8)

    x_d = nc.dram_tensor("x", [T, D], BF16, kind="ExternalInput").ap()
    wq_d = nc.dram_tensor("wq_t", [D, FPC], BF16, kind="ExternalInput").ap()
    wk_d = nc.dram_tensor("wk_t", [D, FPC], BF16, kind="ExternalInput").ap()
    wv_d = nc.dram_tensor("wv_t", [D, FPC], BF16, kind="ExternalInput").ap()
    wo_d = nc.dram_tensor("wo_t", [FPC, D], BF16, kind="ExternalInput").ap()
    onesb_d = nc.dram_tensor("ones_b", [128, 64], BF16, kind="ExternalInput").ap()
    out_d = nc.dram_tensor("po", [T, D], F32, kind="ExternalOutput").ap()
    rscr_d = nc.dram_tensor("rscr", [8, 1024], F32).ap()

    with tile.TileContext(nc) as tc:
        with (
            tc.tile_pool(name="wp", bufs=1) as wp,
            tc.tile_pool(name="qk", bufs=1) as qk,
            tc.tile_pool(name="vp", bufs=1) as vp,
            tc.tile_pool(name="at", bufs=1) as at,
        ):
            qT_sb = qk.tile([128, 2 * T], BF16)   # head-pair hp at cols hp*T
            kT_sb = qk.tile([128, 2 * T], BF16)
            v_sb = vp.tile([128, NKT * HPC * VW], BF16)
            attT_sb = at.tile([128, 2 * T], BF16)

            # ---- phase 1+2: transpose x, projections ----
            with (
                tc.tile_pool(name="xt", bufs=1) as xtp,
                tc.tile_pool(name="xn", bufs=3) as xnp,
                tc.tile_pool(name="ps12", bufs=1, space="PSUM") as ps12,
            ):
                xT_sb = xtp.tile([128, 8 * T], BF16)  # dm chunk kc at cols kc*T
                for kc in range(8):
                    nc.sync.dma_start_transpose(
                        xT_sb[:, kc * T : (kc + 1) * T],
                        x_d[:, kc * 128 : (kc + 1) * 128],
                    )

                # weights (needed later than x, so DMA'd after)
                wq_sb = wp.tile([128, 8 * FPC], BF16)
                nc.sync.dma_start(
                    wq_sb[:].rearrange("p (c f) -> p c f", f=FPC),
                    wq_d.rearrange("(c p) f -> p c f", p=128),
                )
                wk_sb = wp.tile([128, 8 * FPC], BF16)
                nc.sync.dma_start(
                    wk_sb[:].rearrange("p (c f) -> p c f", f=FPC),
                    wk_d.rearrange("(c p) f -> p c f", p=128),
                )
                wv_sb = wp.tile([128, 8 * FPC], BF16)
                nc.sync.dma_start(
                    wv_sb[:].rearrange("p (c f) -> p c f", f=FPC),
                    wv_d.rearrange("(c p) f -> p c f", p=128),
                )
                wo_sb = wp.tile([128, 2 * D], BF16)
                nc.sync.dma_start(
                    wo_sb[:].rearrange("p (c f) -> p c f", f=D),
                    wo_d.rearrange("(c p) f -> p c f", p=128),
                )
                # ones columns of v (every 65th col, offset 64)
                nc.sync.dma_start(
                    v_sb[:].rearrange("p (a b) -> p a b", b=VW)[:, :, 64],
                    onesb_d[:, 0 : NKT * HPC],
                )

                # qT / kT projections: [feat(128=2 heads), tok] blocks
                for tb in range(NQB):
                    q_ps = ps12.tile([128, 512], F32, tag="proj", bufs=2)
                    k_ps = ps12.tile([128, 512], F32, tag="proj", bufs=2)
                    for kc in range(8):
                        nc.tensor.matmul(
                        q_ps[:],
                        wq_sb[:, kc * FPC + 0 * 128 : kc * FPC + (0 + 1) * 128],
                        xT_sb[:, kc * T + tb * 512 : kc * T + (tb + 1) * 512],
                        start=(kc == 0), stop=(kc == 7),
                        )
                    for kc in range(8):
                        nc.tensor.matmul(
                        k_ps[:],
                        wk_sb[:, kc * FPC + 0 * 128 : kc * FPC + (0 + 1) * 128],
                        xT_sb[:, kc * T + tb * 512 : kc * T + (tb + 1) * 512],
                        start=(kc == 0), stop=(kc == 7),
                        )
                    nc.vector.tensor_copy(
                        qT_sb[:, 0 * T + tb * 512 : 0 * T + (tb + 1) * 512], q_ps[:]
                    )
                    nc.vector.tensor_copy(
                        kT_sb[:, 0 * T + tb * 512 : 0 * T + (tb + 1) * 512], k_ps[:]
                    )

                # v projection: natural [tok, feat] tiles
                for tt in range(NKT):
                    v_ps = ps12.tile([128, FPC], F32, tag="vproj", bufs=2)
                    for kc in range(8):
                        nc.tensor.matmul(
                            v_ps[:],
                            xT_sb[:, kc * T + tt * 128 : kc * T + (tt + 1) * 128],
                            wv_sb[:, kc * FPC : (kc + 1) * FPC],
                            start=(kc == 0), stop=(kc == 7),
                        )
                    nc.vector.tensor_copy(
                        v_sb[:].rearrange("p (a b) -> p a b", b=VW)[
                            :, tt * HPC : (tt + 1) * HPC, 0:DH
                        ],
                        v_ps[:].rearrange("p (a b) -> p a b", b=DH),
                    )

                # qT / kT projections for head pair 1
                for tb in range(NQB):
                    q_ps = ps12.tile([128, 512], F32, tag="proj", bufs=2)
                    k_ps = ps12.tile([128, 512], F32, tag="proj", bufs=2)
                    for kc in range(8):
                        nc.tensor.matmul(
                        q_ps[:],
                        wq_sb[:, kc * FPC + 1 * 128 : kc * FPC + (1 + 1) * 128],
                        xT_sb[:, kc * T + tb * 512 : kc * T + (tb + 1) * 512],
                        start=(kc == 0), stop=(kc == 7),
                        )
                    for kc in range(8):
                        nc.tensor.matmul(
                        k_ps[:],
                        wk_sb[:, kc * FPC + 1 * 128 : kc * FPC + (1 + 1) * 128],
                        xT_sb[:, kc * T + tb * 512 : kc * T + (tb + 1) * 512],
                        start=(kc == 0), stop=(kc == 7),
                        )
                    nc.vector.tensor_copy(
                        qT_sb[:, 1 * T + tb * 512 : 1 * T + (tb + 1) * 512], q_ps[:]
                    )
                    nc.vector.tensor_copy(
                        kT_sb[:, 1 * T + tb * 512 : 1 * T + (tb + 1) * 512], k_ps[:]
                    )

            # ---- phase 3            # ---- phase 3: attention + per-block output projection ----
            with (
                tc.tile_pool(name="ep", bufs=4) as ep,
                tc.tile_pool(name="nr", bufs=2) as nrm,
                tc.tile_pool(name="op", bufs=3) as op,
                tc.tile_pool(name="ps3", bufs=1, space="PSUM") as ps3,
            ):
                for qb in range(NQB):
                    for hp in range(2):
                        hA, hB = 2 * hp, 2 * hp + 1
                        oA = ps3.tile([VW, 512], F32, tag="oA", bufs=2)
                        oB = ps3.tile([VW, 512], F32, tag="oB", bufs=2)
                        nkt = 4 * (qb + 1)

                        def attv(e, kt, nkt=nkt, oA=oA, oB=oB, hA=hA, hB=hB):
                            nc.tensor.matmul(
                                oA[:],
                                v_sb[:, (kt * HPC + hA) * VW : (kt * HPC + hA + 1) * VW],
                                e[:, 0:512],
                                start=(kt == 0), stop=(kt == nkt - 1),
                            )
                            nc.tensor.matmul(
                                oB[:],
                                v_sb[:, (kt * HPC + hB) * VW : (kt * HPC + hB + 1) * VW],
                                e[:, 512:1024],
                                start=(kt == 0), stop=(kt == nkt - 1),
                            )

                        prev = None
                        for kt in range(nkt):
                            sAB = ps3.tile([128, 1024], F32, tag="sAB", bufs=2)
                            nc.tensor.matmul(
                                sAB[:, 0:512],
                                kT_sb[0:64, hp * T + kt * 128 : hp * T + (kt + 1) * 128],
                                qT_sb[0:64, hp * T + qb * 512 : hp * T + (qb + 1) * 512],
                                start=True, stop=True, tile_position=(0, 0),
                            )
                            nc.tensor.matmul(
                                sAB[:, 512:1024],
                                kT_sb[64:128, hp * T + kt * 128 : hp * T + (kt + 1) * 128],
                                qT_sb[64:128, hp * T + qb * 512 : hp * T + (qb + 1) * 512],
                                start=True, stop=True, tile_position=(64, 0),
                            )
                            eAB = ep.tile([128, 1024], BF16, tag="eAB")
                            nc.scalar.activation(
                                eAB[:], sAB[:], mybir.ActivationFunctionType.Exp,
                                scale=0.125,
                            )
                            base = qb * 512 - kt * 128
                            if base <= 0:  # diagonal tile: mask k > q
                                nc.gpsimd.affine_select(
                                    out=eAB[:].rearrange("p (h q) -> p h q", q=512),
                                    in_=eAB[:].rearrange("p (h q) -> p h q", q=512),
                                    pattern=[[0, 2], [1, 512]],
                                    compare_op=mybir.AluOpType.is_ge,
                                    fill=0.0, base=base, channel_multiplier=-1,
                                )
                            if prev is not None:
                                attv(*prev)
                            prev = (eAB, kt)
                        attv(*prev)
                        # normalize (qb, hp): pack denoms, reciprocal, bcast, mul
                        srows = nrm.tile([1, 1024], F32, tag="srows")
                        nc.vector.tensor_copy(srows[0:1, 0:512], oA[64:65, :])
                        nc.vector.tensor_copy(srows[0:1, 512:1024], oB[64:65, :])
                        packed = nrm.tile([128, 8], F32, tag="packed")
                        nc.sync.dma_start(
                            packed[:],
                            srows[:].rearrange("r (g e) -> r g e", e=8),
                        )
                        rpacked = nrm.tile([128, 8], F32, tag="rpacked")
                        nc.vector.reciprocal(rpacked[:], packed[:])
                        ridx = qb * 2 + hp
                        rrow_d = rscr_d[ridx : ridx + 1, :]
                        nc.sync.dma_start(
                            rrow_d.rearrange("r (g e) -> r g e", e=8),
                            rpacked[:],
                        )
                        for o_ps, prow, off in ((oA, 0, 0), (oB, 64, 512)):
                            bc = nrm.tile([64, 512], F32, tag="bc")
                            nc.sync.dma_start(
                                bc[:],
                                rrow_d[0:1, off : off + 512].partition_broadcast(64),
                            )
                            nc.vector.tensor_mul(
                                attT_sb[
                                    prow : prow + 64,
                                    hp * T + qb * 512 : hp * T + (qb + 1) * 512,
                                ],
                                o_ps[0:64, :],
                                bc[:],
                            )
                    # output projection for this query block's 4 token tiles
                    for t4 in range(4):
                        tt = qb * 4 + t4
                        o_sb = op.tile([128, D], F32, tag="osb")
                        for nck in range(2):
                            wo_ps = ps3.tile(
                                [128, 512], F32,
                                tag=("oA" if nck == 0 else "oB"), bufs=2,
                            )
                            for hp in range(2):
                                nc.tensor.matmul(
                                    wo_ps[:],
                                    attT_sb[:, hp * T + tt * 128 : hp * T + (tt + 1) * 128],
                                    wo_sb[:, hp * D + nck * 512 : hp * D + (nck + 1) * 512],
                                    start=(hp == 0), stop=(hp == 1),
                                )
                            nc.vector.tensor_copy(
                                o_sb[:, nck * 512 : (nck + 1) * 512], wo_ps[:]
                            )
                        nc.sync.dma_start(out_d[tt * 128 : (tt + 1) * 128, :], o_sb[:])

    nc.compile()
    return nc


def _prep_in_maps(x, Wq, Wk, Wv, Wo):
    x = np.asarray(x, dtype=np.float32)
    bf = ml_dtypes.bfloat16
    Wq = np.asarray(Wq, dtype=np.float32)
    Wk = np.asarray(Wk, dtype=np.float32)
    Wv = np.asarray(Wv, dtype=np.float32)
    Wo = np.asarray(Wo, dtype=np.float32)
    ones_b = np.ones((128, 64), dtype=bf)
    in_maps = []
    for c in range(8):
        b, g = divmod(c, 4)
        sl = slice(g * FPC, (g + 1) * FPC)
        in_maps.append(
            {
                "x": np.ascontiguousarray(x[b]).astype(bf),
                "wq_t": np.ascontiguousarray(Wq[sl, :].T).astype(bf),
                "wk_t": np.ascontiguousarray(Wk[sl, :].T).astype(bf),
                "wv_t": np.ascontiguousarray(Wv[sl, :].T).astype(bf),
                "wo_t": np.ascontiguousarray(Wo[:, sl].T).astype(bf),
                "ones_b": ones_b,
            }
        )
    return in_maps


def _get_nc():
    if "nc" not in _CACHE:
        _CACHE["nc"] = _build()
    return _CACHE["nc"]


def _assemble(results):
    out = np.empty((B, T, D), dtype=np.float32)
    for b in range(B):
        out[b] = (
            results[4 * b]["po"]
            + results[4 * b + 1]["po"]
            + results[4 * b + 2]["po"]
            + results[4 * b + 3]["po"]
        )
    return out


def kernel(x, Wq, Wk, Wv, Wo):
    nc = _get_nc()
    in_maps = _prep_in_maps(x, Wq, Wk, Wv, Wo)
    res = run_bass_kernel_spmd(nc, in_maps, core_ids=list(range(8)))
    return _assemble(res.results)


def kernel_with_trace(x, Wq, Wk, Wv, Wo, **kw):
    nc = _get_nc()
    in_maps = _prep_in_maps(x, Wq, Wk, Wv, Wo)
    res = run_bass_kernel_spmd(nc, in_maps, core_ids=list(range(8)), trace=True, **kw)
    return _assemble(res.results), res
